# revision 2
# baseline (speedup 1.0000x reference)
"""Multi-head attention (B=2, S=2048, D=1024, H=16) on 8 Trainium2 cores.

Sharding: tensor-parallel over heads for QKV+attention (2 heads/core),
then an AllToAll reshards the attention output so each core computes the
output projection for its own 256-row slice of the sequence (both batches).
Host assembles the full output by concatenating the per-core slices.

v2 pipeline (ACT-bound design, ~2.3us per 2-k-chunk step):
  - starting-gun AllGather absorbs inter-core launch skew
  - K(b0) projected ch-outer while x^T streams in; Q tile 0 + V chunks 0,1
    right after -> attention starts ~19us
  - 8 attention units (b,t), each processing BOTH heads: score matmuls for
    head A (partitions 0-63) and head B (64-127) issued adjacently so the
    PE row-tiles them concurrently; exp on ACT [128,1024] per head per step
  - all remaining projection work (V tail, Q tiles 1-3, b1 QKV/V) rides as
    per-step fillers inside the units' PE slack
  - a2a(0) hidden under b1 attention; b0 out-projection + warm dummy MMs
    overlap a2a(1) so the tail collective is not exposed and HAM stays warm

PSUM budget (8 banks): sA [128,1024]x1 = 2, sB [128,1024]x1 = 2,
av [65,512]x3 = 3, o [128,512]x1 = 1.
"""
import os
import sys

sys.path.insert(0, "/opt/trn_rl_repo")

import numpy as np
import ml_dtypes

import concourse.bass as bass
import concourse.tile as tile
from concourse import bacc, mybir
from concourse import bass_utils

B = 2
S = 2048
D = 1024
H = 16
DH = 64
N_CORES = 8
HEADS_PER_CORE = H // N_CORES          # 2
S_SLICE = S // N_CORES                 # 256
N_CH = D // 128                        # 8 contraction chunks
N_QT = S // 512                        # 4 q tiles
N_KC = S // 128                        # 16 k chunks

F32 = mybir.dt.float32
BF16 = mybir.dt.bfloat16

_compiled = None
last_results = None


def _build():
    nc = bacc.Bacc(
        "TRN2",
        target_bir_lowering=False,
        debug=False,
        enable_asserts=True,
        num_devices=N_CORES,
    )

    xtb = nc.dram_tensor("xtb", [B, 128, N_CH, S], BF16, kind="ExternalInput").ap()
    wqt = nc.dram_tensor("wqt", [128, N_CH, 128], BF16, kind="ExternalInput").ap()
    wkt = nc.dram_tensor("wkt", [128, N_CH, 128], BF16, kind="ExternalInput").ap()
    wvt = nc.dram_tensor("wvt", [128, N_CH, 128], BF16, kind="ExternalInput").ap()
    wot = nc.dram_tensor("wot", [128, N_CH, D], BF16, kind="ExternalInput").ap()
    bb = nc.dram_tensor("bb", [128, D], F32, kind="ExternalInput").ap()
    oc = nc.dram_tensor("oc", [B, S_SLICE, D], F32, kind="ExternalOutput").ap()

    EXP = mybir.ActivationFunctionType.Exp
    SCALE = DH ** -0.5

    with tile.TileContext(nc) as tc:
        with (
            tc.tile_pool(name="w", bufs=1) as wp,
            tc.tile_pool(name="qkt", bufs=1) as qktp,
            tc.tile_pool(name="vsb", bufs=1) as vsbp,
            tc.tile_pool(name="xtb", bufs=2) as xtbp,
            tc.tile_pool(name="pt", bufs=2) as ptp,
            tc.tile_pool(name="norm", bufs=2) as normp,
            tc.tile_pool(name="x2", bufs=1) as x2p,
            tc.tile_pool(name="outsb", bufs=2) as outp,
            tc.tile_pool(name="dram", bufs=1, space="DRAM") as dram,
            tc.tile_pool(name="dramsc", bufs=4, space="DRAM") as dramsc,
            tc.tile_pool(name="spsa", bufs=1, space="PSUM") as spsa,
            tc.tile_pool(name="spsb", bufs=1, space="PSUM") as spsb,
            tc.tile_pool(name="avps", bufs=3, space="PSUM") as avps,
            tc.tile_pool(name="ops", bufs=1, space="PSUM") as ops,
        ):
            # ---- weights ----
            wqt_sb = wp.tile([128, N_CH * 128], BF16)
            nc.sync.dma_start(wqt_sb[:], wqt[:].rearrange("p c e -> p (c e)"))
            wkt_sb = wp.tile([128, N_CH * 128], BF16)
            nc.sync.dma_start(wkt_sb[:], wkt[:].rearrange("p c e -> p (c e)"))
            wvt_sb = wp.tile([128, N_CH * 128], BF16)
            nc.sync.dma_start(wvt_sb[:], wvt[:].rearrange("p c e -> p (c e)"))

            # ---- starting gun: tiny AllGather aligns the 8 cores ----
            gun_in = dram.tile([1, 16], F32, name="gun_in")
            gun_out = dram.tile([N_CORES, 16], F32, name="gun_out")
            gun_sb = wp.tile([1, 16], F32)
            nc.gpsimd.memset(gun_sb[:], 0.0)
            nc.sync.dma_start(gun_in[:], gun_sb[:])
            nc.gpsimd.collective_compute(
                "AllGather", mybir.AluOpType.bypass,
                replica_groups=[list(range(N_CORES))],
                ins=[gun_in[:]], outs=[gun_out[:]],
            )

            # ---- PE warmup while x^T DMA streams (HAM clock ramp) ----
            warm = wp.tile([128, 512], BF16)
            nc.gpsimd.memset(warm[:], 0.0)
            for i in range(20):
                wps = ops.tile([128, 512], F32, tag="o", name="wps")
                nc.tensor.matmul(wps[:], lhsT=warm[:, 0:128], rhs=warm[:],
                                 start=True, stop=True)

            xtb_sbs = [None, None]
            Qt, Kt, Vs = [], [], []
            for b in range(B):
                Qt.append(qktp.tile([128, S], BF16, tag=f"qt{b}", name=f"qt{b}"))
                Kt.append(qktp.tile([128, S], BF16, tag=f"kt{b}", name=f"kt{b}"))
                Vs.append(vsbp.tile([128, N_KC * 130], BF16, tag=f"v{b}",
                                    name=f"v{b}"))

            def emit_xtb_load(b):
                t_ = xtbp.tile([128, N_CH * S], BF16, tag="xtb", name="xtb_sb")
                xtb_sbs[b] = t_
                for ch in range(N_CH):
                    nc.sync.dma_start(t_[:, ch * S:(ch + 1) * S], xtb[b, :, ch, :])

            def emit_k_fast(b):
                """ch-outer K projection: 2 passes of 2 tiles, starts as
                x^T chunks land. Pass p uses tag sA/sB psum."""
                for p_ in range(2):
                    k_ps = (spsa if p_ == 0 else spsb).tile(
                        [128, 1024], F32, tag=("sA" if p_ == 0 else "sB"),
                        name="k_ps")
                    for ch in range(N_CH):
                        for j in range(2):
                            t = 2 * p_ + j
                            rhs = xtb_sbs[b][:, ch * S + t * 512:
                                             ch * S + (t + 1) * 512]
                            nc.tensor.matmul(
                                k_ps[:, j * 512:(j + 1) * 512],
                                lhsT=wkt_sb[:, ch * 128:(ch + 1) * 128],
                                rhs=rhs, start=(ch == 0), stop=(ch == N_CH - 1),
                            )
                    nc.vector.tensor_copy(
                        Kt[b][:, p_ * 1024:(p_ + 1) * 1024], k_ps[:])

            def emit_q_half(b, t, half):
                """Half of a Q tile projection (4 ch chunks) on tag-o psum.
                half=0 starts the group, half=1 stops it + copies out."""
                if half == 0:
                    ps_ = ops.tile([128, 512], F32, tag="o", name="q_ps1")
                    _qhold[(b, t)] = ps_
                else:
                    ps_ = _qhold.pop((b, t))
                for ch in range(4 * half, 4 * half + 4):
                    nc.tensor.matmul(
                        ps_[:],
                        lhsT=wqt_sb[:, ch * 128:(ch + 1) * 128],
                        rhs=xtb_sbs[b][:, ch * S + t * 512:
                                       ch * S + (t + 1) * 512],
                        start=(ch == 0), stop=(ch == N_CH - 1),
                        skip_group_check=True,
                    )
                if half == 1:
                    nc.vector.tensor_copy(Qt[b][:, t * 512:(t + 1) * 512], ps_[:])

            def emit_k_half(b, t, half):
                """Half of a K tile projection (ch-inner), for batch 1."""
                if half == 0:
                    ps_ = ops.tile([128, 512], F32, tag="o", name="k_ps1")
                    _khold[(b, t)] = ps_
                else:
                    ps_ = _khold.pop((b, t))
                for ch in range(4 * half, 4 * half + 4):
                    nc.tensor.matmul(
                        ps_[:],
                        lhsT=wkt_sb[:, ch * 128:(ch + 1) * 128],
                        rhs=xtb_sbs[b][:, ch * S + t * 512:
                                       ch * S + (t + 1) * 512],
                        start=(ch == 0), stop=(ch == N_CH - 1),
                        skip_group_check=True,
                    )
                if half == 1:
                    nc.vector.tensor_copy(Kt[b][:, t * 512:(t + 1) * 512], ps_[:])

            _qhold, _khold = {}, {}

            def emit_v(b, st):
                """One V chunk (128 seq rows, both heads) -> Vs layout."""
                v_ps = ops.tile([128, 512], F32, tag="o", name="v_ps")
                for ch in range(N_CH):
                    nc.tensor.matmul(
                        v_ps[:, 0:128],
                        lhsT=xtb_sbs[b][:, ch * S + st * 128:
                                        ch * S + (st + 1) * 128],
                        rhs=wvt_sb[:, ch * 128:(ch + 1) * 128],
                        start=(ch == 0), stop=(ch == N_CH - 1),
                        skip_group_check=True,
                    )
                dst = Vs[b][:].rearrange("p (c o) -> p c o", o=65)[
                    :, 2 * st:2 * st + 2, 0:64
                ]
                nc.vector.tensor_copy(
                    dst, v_ps[:, 0:128].rearrange("p (h e) -> p h e", e=64)
                )

            a2a_in = [dram.tile([N_CORES, 128, S_SLICE], BF16, tag=f"a2ai{b}",
                                name=f"a2ai{b}") for b in range(B)]
            a2a_out = [dram.tile([N_CORES, 128, S_SLICE], BF16, tag=f"a2ao{b}",
                                 name=f"a2ao{b}") for b in range(B)]

            def normalize_and_emit(b, h, t, av):
                """softmax denominator -> reciprocal -> broadcast -> a2a_in."""
                hp = slice(h * 64, (h + 1) * 64)
                den_sb = normp.tile([1, 512], F32, tag="dsb", name="den_sb")
                nc.vector.tensor_copy(den_sb[:], av[64:65, :])
                den_d = dramsc.tile([512], F32, tag="dend", name="den_d")
                nc.sync.dma_start(
                    den_d[:].rearrange("(a q) -> a q", a=1), den_sb[:])
                den64 = normp.tile([64, 8], F32, tag="d64", name="den64")
                nc.sync.dma_start(
                    den64[:], den_d[:].rearrange("(p q) -> p q", p=64))
                rec64 = normp.tile([64, 8], F32, tag="r64", name="rec64")
                nc.vector.reciprocal(rec64[:], den64[:])
                rsc = dramsc.tile([512], F32, tag="rsc", name="rsc")
                nc.sync.dma_start(
                    rsc[:].rearrange("(p q) -> p q", p=64), rec64[:])
                bcast = normp.tile([64, 512], F32, tag="bc", name="bcast")
                nc.sync.dma_start(
                    bcast[:],
                    rsc[:].rearrange("(a q) -> a q", a=1).broadcast_to([64, 512]),
                )
                o_sb = normp.tile([64, 512], BF16, tag="ob", name="o_sb")
                nc.vector.tensor_mul(o_sb[:], av[0:64, :], bcast[:])
                for j in range(2):
                    nc.sync.dma_start(
                        a2a_in[b][2 * t + j, hp, :],
                        o_sb[:, j * S_SLICE:(j + 1) * S_SLICE],
                    )

            def att_unit(b, t, fillers):
                """Attention for BOTH heads of (b, q-tile t).

                Per step cc (2 k-chunks): filler, score MMs interleaved
                A/B (row-tile concurrency), exp A, exp B, AV MMs.
                fillers: list of 8 lists of closures, one list per cc.
                """
                avA = avps.tile([65, 512], F32, tag="av", name="avA")
                avB = avps.tile([65, 512], F32, tag="av", name="avB")
                qs = slice(t * 512, (t + 1) * 512)
                hA, hB = slice(0, 64), slice(64, 128)
                for cc in range(N_KC // 2):
                    for f in fillers[cc]:
                        f()
                    sA = spsa.tile([128, 1024], F32, tag="sA", name="sA")
                    sB = spsb.tile([128, 1024], F32, tag="sB", name="sB")
                    for j in range(2):
                        c = 2 * cc + j
                        ks = slice(c * 128, (c + 1) * 128)
                        nc.tensor.matmul(
                            sA[:, j * 512:(j + 1) * 512],
                            lhsT=Kt[b][hA, ks], rhs=Qt[b][hA, qs],
                            start=True, stop=True,
                        )
                        nc.tensor.matmul(
                            sB[:, j * 512:(j + 1) * 512],
                            lhsT=Kt[b][hB, ks], rhs=Qt[b][hB, qs],
                            start=True, stop=True,
                        )
                    pA = ptp.tile([128, 1024], BF16, tag="pA", name="pA")
                    nc.scalar.activation(pA[:], sA[:], EXP, scale=SCALE)
                    pB = ptp.tile([128, 1024], BF16, tag="pB", name="pB")
                    nc.scalar.activation(pB[:], sB[:], EXP, scale=SCALE)
                    for j in range(2):
                        c = 2 * cc + j
                        nc.tensor.matmul(
                            avA[:],
                            lhsT=Vs[b][:, c * 130:c * 130 + 65],
                            rhs=pA[:, j * 512:(j + 1) * 512],
                            start=(c == 0), stop=(c == N_KC - 1),
                            skip_group_check=True,
                        )
                    for j in range(2):
                        c = 2 * cc + j
                        nc.tensor.matmul(
                            avB[:],
                            lhsT=Vs[b][:, c * 130 + 65:c * 130 + 130],
                            rhs=pB[:, j * 512:(j + 1) * 512],
                            start=(c == 0), stop=(c == N_KC - 1),
                            skip_group_check=True,
                        )
                normalize_and_emit(b, 0, t, avA)
                normalize_and_emit(b, 1, t, avB)

            def emit_a2a(b):
                nc.gpsimd.collective_compute(
                    "AllToAll", mybir.AluOpType.bypass,
                    replica_groups=[list(range(N_CORES))],
                    ins=[a2a_in[b][:]], outs=[a2a_out[b][:]],
                )

            x2_tiles = {}

            def emit_x2_loads(b):
                x2 = []
                for i in range(N_CH):
                    x2_sb = x2p.tile([128, S_SLICE], BF16, tag=f"x2_{b}_{i}",
                                     name=f"x2_{b}_{i}")
                    nc.sync.dma_start(x2_sb[:], a2a_out[b][i])
                    x2.append(x2_sb)
                x2_tiles[b] = x2

            def emit_outproj_piece(b, st, et, wot_sb, bb_sb):
                o_ps = ops.tile([128, 512], F32, tag="o", name="o_ps")
                for ch in range(N_CH):
                    nc.tensor.matmul(
                        o_ps[:],
                        lhsT=x2_tiles[b][ch][:, st * 128:(st + 1) * 128],
                        rhs=wot_sb[:, ch * D + et * 512:ch * D + (et + 1) * 512],
                        start=(ch == 0), stop=(ch == N_CH - 1),
                    )
                out_sb = outp.tile([128, 512], F32, tag="osb", name="out_sb")
                nc.vector.tensor_add(
                    out_sb[:], o_ps[:], bb_sb[:, et * 512:(et + 1) * 512])
                nc.sync.dma_start(
                    oc[b, st * 128:(st + 1) * 128, et * 512:(et + 1) * 512],
                    out_sb[:],
                )

            def emit_warm_dummy(n):
                for _ in range(n):
                    wps = ops.tile([128, 512], F32, tag="o", name="wdum")
                    nc.tensor.matmul(wps[:], lhsT=warm[:, 0:128], rhs=warm[:],
                                     start=True, stop=True)

            # ================= pipeline =================
            emit_xtb_load(0)
            ones0 = Vs[0][:].rearrange("p (c o) -> p c o", o=65)[:, :, 64:65]
            nc.gpsimd.memset(ones0, 1.0)
            ones1 = Vs[1][:].rearrange("p (c o) -> p c o", o=65)[:, :, 64:65]
            nc.gpsimd.memset(ones1, 1.0)

            emit_k_fast(0)                      # K(b0) rides the x DMA
            emit_q_half(0, 0, 0)                # Q tile 0
            emit_q_half(0, 0, 1)
            emit_v(0, 0)                        # V chunks 0,1
            emit_v(0, 1)
            emit_xtb_load(1)
            wot_sb = wp.tile([128, N_CH * D], BF16)
            nc.sync.dma_start(wot_sb[:], wot[:].rearrange("p c e -> p (c e)"))
            bb_sb = wp.tile([128, D], F32)
            nc.sync.dma_start(bb_sb[:], bb[:])

            # filler schedule: 8 units x 8 cc slots, hand-placed so every
            # tensor is emitted before its first consumer (see constraints
            # in unit comments).
            F = lambda fn, *a: (lambda: fn(*a))
            sched = {
                # unit (0,0): V(b0) pairs progressively (av cc needs chunks
                # 2cc,2cc+1), then Q tile 1 (needed by unit (0,1))
                (0, 0): [[F(emit_v, 0, 2), F(emit_v, 0, 3)],
                         [F(emit_v, 0, 4), F(emit_v, 0, 5)],
                         [F(emit_v, 0, 6), F(emit_v, 0, 7)],
                         [F(emit_v, 0, 8), F(emit_v, 0, 9)],
                         [F(emit_v, 0, 10), F(emit_v, 0, 11)],
                         [F(emit_v, 0, 12), F(emit_v, 0, 13)],
                         [F(emit_v, 0, 14), F(emit_v, 0, 15)],
                         [F(emit_q_half, 0, 1, 0), F(emit_q_half, 0, 1, 1)]],
                # unit (0,1): Q tiles 2,3 (b0), then K(b1) tiles 0,1
                (0, 1): [[F(emit_q_half, 0, 2, 0)], [F(emit_q_half, 0, 2, 1)],
                         [F(emit_q_half, 0, 3, 0)], [F(emit_q_half, 0, 3, 1)],
                         [F(emit_k_half, 1, 0, 0)], [F(emit_k_half, 1, 0, 1)],
                         [F(emit_k_half, 1, 1, 0)], [F(emit_k_half, 1, 1, 1)]],
                # unit (0,2): K(b1) tiles 2,3, Q(b1) tile 0, V(b1) 0,1
                (0, 2): [[F(emit_k_half, 1, 2, 0)], [F(emit_k_half, 1, 2, 1)],
                         [F(emit_k_half, 1, 3, 0)], [F(emit_k_half, 1, 3, 1)],
                         [F(emit_q_half, 1, 0, 0)], [F(emit_q_half, 1, 0, 1)],
                         [F(emit_v, 1, 0)], [F(emit_v, 1, 1)]],
                # unit (0,3): V(b1) 2..9
                (0, 3): [[F(emit_v, 1, 2)], [F(emit_v, 1, 3)],
                         [F(emit_v, 1, 4)], [F(emit_v, 1, 5)],
                         [F(emit_v, 1, 6)], [F(emit_v, 1, 7)],
                         [F(emit_v, 1, 8)], [F(emit_v, 1, 9)]],
                # unit (1,0): V(b1) 10..15 (consumed before needed: av cc5
                # needs chunks 10,11 - emitted at cc0,cc1), Q(b1) tile 1
                (1, 0): [[F(emit_v, 1, 10)], [F(emit_v, 1, 11)],
                         [F(emit_v, 1, 12)], [F(emit_v, 1, 13)],
                         [F(emit_v, 1, 14)], [F(emit_v, 1, 15)],
                         [F(emit_q_half, 1, 1, 0)], [F(emit_q_half, 1, 1, 1)]],
                # unit (1,1): Q(b1) tiles 2,3
                (1, 1): [[F(emit_q_half, 1, 2, 0)], [F(emit_q_half, 1, 2, 1)],
                         [F(emit_q_half, 1, 3, 0)], [F(emit_q_half, 1, 3, 1)],
                         [], [], [], []],
                (1, 2): [[], [], [], [], [], [], [], []],
                (1, 3): [[], [], [], [], [], [], [], []],
            }

            for t in range(N_QT):
                att_unit(0, t, sched[(0, t)])
            emit_a2a(0)
            for t in range(N_QT):
                att_unit(1, t, sched[(1, t)])
                if t == 1:
                    emit_x2_loads(0)   # a2a(0) long done by now
            emit_a2a(1)

            # b0 out-projection + warm dummies overlap a2a(1)
            pieces = [(st, et) for st in range(S_SLICE // 128)
                      for et in range(D // 512)]
            for i, (st, et) in enumerate(pieces):
                emit_outproj_piece(0, st, et, wot_sb, bb_sb)
                emit_warm_dummy(6)
            emit_x2_loads(1)
            for st, et in pieces:
                emit_outproj_piece(1, st, et, wot_sb, bb_sb)

    nc.compile()
    return nc


def _prep_chunked(a_t):
    """[Din, E] (already transposed) -> [128, Din//128, E] SBUF-chunk layout."""
    din, e = a_t.shape
    return np.ascontiguousarray(
        a_t.reshape(din // 128, 128, e).transpose(1, 0, 2)
    )


def kernel(x, w_qkv, w_out, b_out):
    global _compiled, last_results
    if _compiled is None:
        _compiled = _build()
    nc = _compiled

    x = np.asarray(x, dtype=np.float32)
    w_qkv = np.asarray(w_qkv, dtype=np.float32)
    w_out = np.asarray(w_out, dtype=np.float32)
    b_out = np.asarray(b_out, dtype=np.float32)

    # x^T in chunk layout: [B, 128, N_CH, S], bf16
    xt_full = x.transpose(0, 2, 1)  # [B, D, S]
    xtb_prep = np.ascontiguousarray(
        xt_full.reshape(B, N_CH, 128, S).transpose(0, 2, 1, 3)
    ).astype(ml_dtypes.bfloat16)

    wot_prep = _prep_chunked(np.ascontiguousarray(w_out.T)).astype(ml_dtypes.bfloat16)
    bb_np = np.ascontiguousarray(np.broadcast_to(b_out, (128, D)))

    in_maps = []
    for c in range(N_CORES):
        hA, hB = HEADS_PER_CORE * c, HEADS_PER_CORE * c + 1
        rows = np.r_[hA * DH:(hA + 1) * DH, hB * DH:(hB + 1) * DH]
        wq = w_qkv[rows, :]               # [128, D]
        wk = w_qkv[D + rows, :]
        wv = w_qkv[2 * D + rows, :]
        in_maps.append({
            "xtb": xtb_prep,
            "wqt": _prep_chunked(np.ascontiguousarray(wq.T)).astype(ml_dtypes.bfloat16),
            "wkt": _prep_chunked(np.ascontiguousarray(wk.T)).astype(ml_dtypes.bfloat16),
            "wvt": _prep_chunked(np.ascontiguousarray(wv.T)).astype(ml_dtypes.bfloat16),
            "wot": wot_prep,
            "bb": bb_np,
        })

    last_results = bass_utils.run_bass_kernel_spmd(
        nc, in_maps, core_ids=list(range(N_CORES))
    )
    out = np.concatenate(
        [last_results.results[c]["oc"] for c in range(N_CORES)], axis=1
    )
    return out


# revision 6
# speedup vs baseline: 1.0462x; 1.0462x over previous
"""Multi-head attention (B=2, S=2048, D=1024, H=16) on 8 Trainium2 cores.

Sharding: tensor-parallel over heads for QKV+attention (2 heads/core).
Each batch's attention output is resharded by TWO AllToAlls (one per
1024-row half of the sequence, [8,128,128] bf16 each); core c receives
q rows [1024k + 128c, +128) of half k and computes the output projection
for them. Host reassembles by interleaving the 128-row blocks.

v3 pipeline (ACT-bound design, ~1.15us per k-chunk):
  - starting-gun AllGather absorbs inter-core launch skew
  - K(b0) projected ch-outer while x^T streams in; Q tile 0 + V chunks
    0..7 after -> attention starts ~22us
  - 8 attention units (b,t): per k-chunk, BOTH heads' score matmuls land
    in ONE [128,1024] psum tile (head A cols 0:512 at rows 0-63, head B
    cols 512:1024 at rows 64-127) so they issue adjacently and row-tile
    concurrently; one exp [128,1024] per chunk; AV per head
  - remaining projection work (V tails, Q tiles, b1 QKV/V, early
    out-projections) rides a budget-weighted filler queue inside the
    units' PE slack
  - the 4 half-AllToAlls overlap compute; only the last one plus one
    out-projection half is exposed, with warm dummy MMs covering it

PSUM budget (8 banks): s [128,1024]x2 = 4, av [65,512]x3 = 3,
o [128,512]x1 = 1.
"""
import os
import sys

sys.path.insert(0, "/opt/trn_rl_repo")

import numpy as np
import ml_dtypes

import concourse.bass as bass
import concourse.tile as tile
from concourse import bacc, mybir
from concourse import bass_utils

B = 2
S = 2048
D = 1024
H = 16
DH = 64
N_CORES = 8
HEADS_PER_CORE = H // N_CORES          # 2
S_SLICE = S // N_CORES                 # 256
N_CH = D // 128                        # 8 contraction chunks
N_QT = S // 512                        # 4 q tiles
N_KC = S // 128                        # 16 k chunks

F32 = mybir.dt.float32
BF16 = mybir.dt.bfloat16

_compiled = None
last_results = None


def _build():
    nc = bacc.Bacc(
        "TRN2",
        target_bir_lowering=False,
        debug=False,
        enable_asserts=True,
        num_devices=N_CORES,
    )

    xtb = nc.dram_tensor("xtb", [B, 128, N_CH, S], BF16, kind="ExternalInput").ap()
    wqt = nc.dram_tensor("wqt", [128, N_CH, 128], BF16, kind="ExternalInput").ap()
    wkt = nc.dram_tensor("wkt", [128, N_CH, 128], BF16, kind="ExternalInput").ap()
    wvt = nc.dram_tensor("wvt", [128, N_CH, 128], BF16, kind="ExternalInput").ap()
    wot = nc.dram_tensor("wot", [128, N_CH, D], BF16, kind="ExternalInput").ap()
    bb = nc.dram_tensor("bb", [128, D], F32, kind="ExternalInput").ap()
    # oc rows: [half k=0 (q rows 128c..), half k=1 (q rows 1024+128c..)]
    oc = nc.dram_tensor("oc", [B, S_SLICE, D], F32, kind="ExternalOutput").ap()

    EXP = mybir.ActivationFunctionType.Exp
    SCALE = DH ** -0.5

    with tile.TileContext(nc) as tc:
        with (
            tc.tile_pool(name="w", bufs=1) as wp,
            tc.tile_pool(name="qkt", bufs=1) as qktp,
            tc.tile_pool(name="vsb", bufs=1) as vsbp,
            tc.tile_pool(name="xtb", bufs=2) as xtbp,
            tc.tile_pool(name="pt", bufs=2) as ptp,
            tc.tile_pool(name="norm", bufs=2) as normp,
            tc.tile_pool(name="x2", bufs=1) as x2p,
            tc.tile_pool(name="outsb", bufs=2) as outp,
            tc.tile_pool(name="dram", bufs=1, space="DRAM") as dram,
            tc.tile_pool(name="dramsc", bufs=4, space="DRAM") as dramsc,
            tc.tile_pool(name="sps", bufs=2, space="PSUM") as sps,
            tc.tile_pool(name="avps", bufs=3, space="PSUM") as avps,
            tc.tile_pool(name="ops", bufs=1, space="PSUM") as ops,
        ):
            # ---- weights ----
            wqt_sb = wp.tile([128, N_CH * 128], BF16)
            nc.sync.dma_start(wqt_sb[:], wqt[:].rearrange("p c e -> p (c e)"))
            wkt_sb = wp.tile([128, N_CH * 128], BF16)
            nc.sync.dma_start(wkt_sb[:], wkt[:].rearrange("p c e -> p (c e)"))
            wvt_sb = wp.tile([128, N_CH * 128], BF16)
            nc.sync.dma_start(wvt_sb[:], wvt[:].rearrange("p c e -> p (c e)"))

            # ---- starting gun: tiny AllGather aligns the 8 cores ----
            gun_in = dram.tile([1, 16], F32, name="gun_in")
            gun_out = dram.tile([N_CORES, 16], F32, name="gun_out")
            gun_sb = wp.tile([1, 16], F32)
            nc.gpsimd.memset(gun_sb[:], 0.0)
            nc.sync.dma_start(gun_in[:], gun_sb[:])
            nc.gpsimd.collective_compute(
                "AllGather", mybir.AluOpType.bypass,
                replica_groups=[list(range(N_CORES))],
                ins=[gun_in[:]], outs=[gun_out[:]],
            )

            # ---- PE warmup while x^T DMA streams (HAM clock ramp) ----
            warm = wp.tile([128, 512], BF16)
            nc.gpsimd.memset(warm[:], 0.0)
            for i in range(20):
                wps = ops.tile([128, 512], F32, tag="o", name="wps")
                nc.tensor.matmul(wps[:], lhsT=warm[:, 0:128], rhs=warm[:],
                                 start=True, stop=True)

            xtb_sbs = [None, None]
            Qt, Kt, Vs = [], [], []
            for b in range(B):
                Qt.append(qktp.tile([128, S], BF16, tag=f"qt{b}", name=f"qt{b}"))
                Kt.append(qktp.tile([128, S], BF16, tag=f"kt{b}", name=f"kt{b}"))
                Vs.append(vsbp.tile([128, N_KC * 130], BF16, tag=f"v{b}",
                                    name=f"v{b}"))

            def emit_xtb_load(b):
                t_ = xtbp.tile([128, N_CH * S], BF16, tag="xtb", name="xtb_sb")
                xtb_sbs[b] = t_
                for ch in range(N_CH):
                    nc.sync.dma_start(t_[:, ch * S:(ch + 1) * S], xtb[b, :, ch, :])

            def emit_k_fast(b):
                """ch-outer K projection: 2 passes of 2 tiles, starts as
                x^T chunks land. Uses the s-tag [128,1024] psum."""
                for p_ in range(2):
                    k_ps = sps.tile([128, 1024], F32, tag="s", name="k_ps")
                    for ch in range(N_CH):
                        for j in range(2):
                            t = 2 * p_ + j
                            rhs = xtb_sbs[b][:, ch * S + t * 512:
                                             ch * S + (t + 1) * 512]
                            nc.tensor.matmul(
                                k_ps[:, j * 512:(j + 1) * 512],
                                lhsT=wkt_sb[:, ch * 128:(ch + 1) * 128],
                                rhs=rhs, start=(ch == 0), stop=(ch == N_CH - 1),
                            )
                    nc.vector.tensor_copy(
                        Kt[b][:, p_ * 1024:(p_ + 1) * 1024], k_ps[:])

            _hold = {}

            def emit_proj_half(w_sb, dst, b, t, half):
                """Half of a Q/K tile projection (4 ch chunks) on tag-o psum.
                half=0 starts the group, half=1 stops it + copies out."""
                key = (id(w_sb), b, t)
                if half == 0:
                    ps_ = ops.tile([128, 512], F32, tag="o", name="pj_ps")
                    _hold[key] = ps_
                else:
                    ps_ = _hold.pop(key)
                for ch in range(4 * half, 4 * half + 4):
                    nc.tensor.matmul(
                        ps_[:],
                        lhsT=w_sb[:, ch * 128:(ch + 1) * 128],
                        rhs=xtb_sbs[b][:, ch * S + t * 512:
                                       ch * S + (t + 1) * 512],
                        start=(ch == 0), stop=(ch == N_CH - 1),
                        skip_group_check=True,
                    )
                if half == 1:
                    nc.vector.tensor_copy(dst[b][:, t * 512:(t + 1) * 512],
                                          ps_[:])

            def emit_v(b, st):
                """One V chunk (128 seq rows, both heads) -> Vs layout."""
                v_ps = ops.tile([128, 512], F32, tag="o", name="v_ps")
                for ch in range(N_CH):
                    nc.tensor.matmul(
                        v_ps[:, 0:128],
                        lhsT=xtb_sbs[b][:, ch * S + st * 128:
                                        ch * S + (st + 1) * 128],
                        rhs=wvt_sb[:, ch * 128:(ch + 1) * 128],
                        start=(ch == 0), stop=(ch == N_CH - 1),
                        skip_group_check=True,
                    )
                dst = Vs[b][:].rearrange("p (c o) -> p c o", o=65)[
                    :, 2 * st:2 * st + 2, 0:64
                ]
                nc.vector.tensor_copy(
                    dst, v_ps[:, 0:128].rearrange("p (h e) -> p h e", e=64)
                )

            # a2a halves: index hx = b*2 + k, k = seq half
            a2a_in = [dram.tile([N_CORES, 128, 128], BF16, tag=f"a2ai{hx}",
                                name=f"a2ai{hx}") for hx in range(2 * B)]
            a2a_out = [dram.tile([N_CORES, 128, 128], BF16, tag=f"a2ao{hx}",
                                 name=f"a2ao{hx}") for hx in range(2 * B)]

            def normalize_and_emit(b, h, t, av):
                """Copy av out of PSUM immediately (frees the bank), then
                denominator -> reciprocal -> broadcast -> a2a_in."""
                hp = slice(h * 64, (h + 1) * 64)
                hx = b * 2 + t // 2
                av_sb = normp.tile([65, 512], F32, tag="avsb", name="av_sb")
                nc.vector.tensor_copy(av_sb[:], av[:])
                den_d = dramsc.tile([512], F32, tag="dend", name="den_d")
                nc.sync.dma_start(
                    den_d[:].rearrange("(a q) -> a q", a=1), av_sb[64:65, :])
                den64 = normp.tile([64, 8], F32, tag="d64", name="den64")
                nc.sync.dma_start(
                    den64[:], den_d[:].rearrange("(p q) -> p q", p=64))
                rec64 = normp.tile([64, 8], F32, tag="r64", name="rec64")
                nc.vector.reciprocal(rec64[:], den64[:])
                rsc = dramsc.tile([512], F32, tag="rsc", name="rsc")
                nc.sync.dma_start(
                    rsc[:].rearrange("(p q) -> p q", p=64), rec64[:])
                bcast = normp.tile([64, 512], F32, tag="bc", name="bcast")
                nc.sync.dma_start(
                    bcast[:],
                    rsc[:].rearrange("(a q) -> a q", a=1).broadcast_to([64, 512]),
                )
                o_sb = normp.tile([64, 512], BF16, tag="ob", name="o_sb")
                nc.vector.tensor_mul(o_sb[:], av_sb[0:64, :], bcast[:])
                # unit t covers global q rows [512t, 512t+512) = dest blocks
                # c = 4*(t%2) .. +4 of half k=t//2 (128 cols each)
                for j in range(4):
                    c = 4 * (t % 2) + j
                    nc.sync.dma_start(
                        a2a_in[hx][c, hp, :],
                        o_sb[:, j * 128:(j + 1) * 128],
                    )

            # ---- filler queue: closures with ns cost estimates ----
            fq = []

            def pop_fillers(budget):
                while fq and budget > 0:
                    cost, fn = fq.pop(0)
                    fn()
                    budget -= cost

            def att_unit(b, t):
                """Attention for BOTH heads of (b, q-tile t)."""
                avA = avps.tile([65, 512], F32, tag="av", name="avA")
                avB = avps.tile([65, 512], F32, tag="av", name="avB")
                qs = slice(t * 512, (t + 1) * 512)
                hA, hB = slice(0, 64), slice(64, 128)
                for c in range(N_KC):
                    pop_fillers(500)
                    ks = slice(c * 128, (c + 1) * 128)
                    s = sps.tile([128, 1024], F32, tag="s", name="s")
                    nc.tensor.matmul(
                        s[:, 0:512],
                        lhsT=Kt[b][hA, ks], rhs=Qt[b][hA, qs],
                        start=True, stop=True,
                    )
                    nc.tensor.matmul(
                        s[:, 512:1024],
                        lhsT=Kt[b][hB, ks], rhs=Qt[b][hB, qs],
                        start=True, stop=True,
                    )
                    p = ptp.tile([128, 1024], BF16, tag="p", name="p")
                    nc.scalar.activation(p[:], s[:], EXP, scale=SCALE)
                    nc.tensor.matmul(
                        avA[:],
                        lhsT=Vs[b][:, c * 130:c * 130 + 65],
                        rhs=p[:, 0:512],
                        start=(c == 0), stop=(c == N_KC - 1),
                        skip_group_check=True,
                    )
                    nc.tensor.matmul(
                        avB[:],
                        lhsT=Vs[b][:, c * 130 + 65:c * 130 + 130],
                        rhs=p[:, 512:1024],
                        start=(c == 0), stop=(c == N_KC - 1),
                        skip_group_check=True,
                    )
                normalize_and_emit(b, 0, t, avA)
                normalize_and_emit(b, 1, t, avB)

            def emit_a2a(hx):
                nc.gpsimd.collective_compute(
                    "AllToAll", mybir.AluOpType.bypass,
                    replica_groups=[list(range(N_CORES))],
                    ins=[a2a_in[hx][:]], outs=[a2a_out[hx][:]],
                )

            x2_tiles = {}

            def emit_x2_loads(hx):
                x2 = []
                for i in range(N_CH):
                    x2_sb = x2p.tile([128, 128], BF16, tag=f"x2_{hx}_{i}",
                                     name=f"x2_{hx}_{i}")
                    nc.sync.dma_start(x2_sb[:], a2a_out[hx][i])
                    x2.append(x2_sb)
                x2_tiles[hx] = x2

            def emit_outproj_half(hx, et, half, wot_sb, bb_sb):
                b, k = hx // 2, hx % 2
                key = ("op", hx, et)
                if half == 0:
                    o_ps = ops.tile([128, 512], F32, tag="o", name="o_ps")
                    _hold[key] = o_ps
                else:
                    o_ps = _hold.pop(key)
                for ch in range(4 * half, 4 * half + 4):
                    nc.tensor.matmul(
                        o_ps[:],
                        lhsT=x2_tiles[hx][ch][:],
                        rhs=wot_sb[:, ch * D + et * 512:ch * D + (et + 1) * 512],
                        start=(ch == 0), stop=(ch == N_CH - 1),
                        skip_group_check=True,
                    )
                if half == 0:
                    return
                out_sb = outp.tile([128, 512], F32, tag="osb", name="out_sb")
                nc.vector.tensor_add(
                    out_sb[:], o_ps[:], bb_sb[:, et * 512:(et + 1) * 512])
                nc.sync.dma_start(
                    oc[b, k * 128:(k + 1) * 128, et * 512:(et + 1) * 512],
                    out_sb[:],
                )

            def emit_warm_dummy(n):
                for _ in range(n):
                    wps = ops.tile([128, 512], F32, tag="o", name="wdum")
                    nc.tensor.matmul(wps[:], lhsT=warm[:, 0:128], rhs=warm[:],
                                     start=True, stop=True)

            # ================= pipeline =================
            emit_xtb_load(0)
            ones0 = Vs[0][:].rearrange("p (c o) -> p c o", o=65)[:, :, 64:65]
            nc.gpsimd.memset(ones0, 1.0)
            ones1 = Vs[1][:].rearrange("p (c o) -> p c o", o=65)[:, :, 64:65]
            nc.gpsimd.memset(ones1, 1.0)

            emit_k_fast(0)                      # K(b0) rides the x DMA
            emit_proj_half(wqt_sb, Qt, 0, 0, 0)  # Q tile 0
            emit_proj_half(wqt_sb, Qt, 0, 0, 1)
            for st in range(8):                 # V(b0) chunks 0..7
                emit_v(0, st)
            emit_xtb_load(1)
            wot_sb = wp.tile([128, N_CH * D], BF16)
            nc.sync.dma_start(wot_sb[:], wot[:].rearrange("p c e -> p (c e)"))
            bb_sb = wp.tile([128, D], F32)
            nc.sync.dma_start(bb_sb[:], bb[:])

            F = lambda cost, fn, *a: (cost, (lambda: fn(*a)))
            # V(b0) tail first (pacing: chunk c needed at unit(0,0) slot c;
            # emitted with >=8-chunk lookahead), then b0 Q tiles, then b1
            # projections. ~36us total vs ~64us of slack.
            fq += [F(1000, emit_v, 0, st) for st in range(8, 16)]
            for t in range(1, 4):
                fq += [F(900, emit_proj_half, wqt_sb, Qt, 0, t, 0),
                       F(900, emit_proj_half, wqt_sb, Qt, 0, t, 1)]
            for t in range(4):
                fq += [F(900, emit_proj_half, wkt_sb, Kt, 1, t, 0),
                       F(900, emit_proj_half, wkt_sb, Kt, 1, t, 1)]
            fq += [F(900, emit_proj_half, wqt_sb, Qt, 1, 0, 0),
                   F(900, emit_proj_half, wqt_sb, Qt, 1, 0, 1)]
            fq += [F(1000, emit_v, 1, st) for st in range(16)]
            for t in range(1, 4):
                fq += [F(900, emit_proj_half, wqt_sb, Qt, 1, t, 0),
                       F(900, emit_proj_half, wqt_sb, Qt, 1, t, 1)]

            att_unit(0, 0)
            att_unit(0, 1)
            emit_a2a(0)                         # b0 half 0
            att_unit(0, 2)
            emit_x2_loads(0)
            fq += [F(1100, emit_outproj_half, 0, et, h, wot_sb, bb_sb)
                   for et in range(2) for h in range(2)]
            att_unit(0, 3)
            emit_a2a(1)                         # b0 half 1
            att_unit(1, 0)
            emit_x2_loads(1)
            fq += [F(1100, emit_outproj_half, 1, et, h, wot_sb, bb_sb)
                   for et in range(2) for h in range(2)]
            att_unit(1, 1)
            emit_a2a(2)                         # b1 half 0
            att_unit(1, 2)
            emit_x2_loads(2)
            fq += [F(1100, emit_outproj_half, 2, et, h, wot_sb, bb_sb)
                   for et in range(2) for h in range(2)]
            att_unit(1, 3)
            emit_a2a(3)                         # b1 half 1

            # drain any unconsumed fillers, then overlap a2a(3)
            while fq:
                pop_fillers(10000)
            emit_warm_dummy(30)
            emit_x2_loads(3)
            for et in range(2):
                emit_outproj_half(3, et, 0, wot_sb, bb_sb)
                emit_outproj_half(3, et, 1, wot_sb, bb_sb)

    nc.compile()
    return nc


def _prep_chunked(a_t):
    """[Din, E] (already transposed) -> [128, Din//128, E] SBUF-chunk layout."""
    din, e = a_t.shape
    return np.ascontiguousarray(
        a_t.reshape(din // 128, 128, e).transpose(1, 0, 2)
    )


def kernel(x, w_qkv, w_out, b_out):
    global _compiled, last_results
    if _compiled is None:
        _compiled = _build()
    nc = _compiled

    x = np.asarray(x, dtype=np.float32)
    w_qkv = np.asarray(w_qkv, dtype=np.float32)
    w_out = np.asarray(w_out, dtype=np.float32)
    b_out = np.asarray(b_out, dtype=np.float32)

    # x^T in chunk layout: [B, 128, N_CH, S], bf16
    xt_full = x.transpose(0, 2, 1)  # [B, D, S]
    xtb_prep = np.ascontiguousarray(
        xt_full.reshape(B, N_CH, 128, S).transpose(0, 2, 1, 3)
    ).astype(ml_dtypes.bfloat16)

    wot_prep = _prep_chunked(np.ascontiguousarray(w_out.T)).astype(ml_dtypes.bfloat16)
    bb_np = np.ascontiguousarray(np.broadcast_to(b_out, (128, D)))

    in_maps = []
    for c in range(N_CORES):
        hA, hB = HEADS_PER_CORE * c, HEADS_PER_CORE * c + 1
        rows = np.r_[hA * DH:(hA + 1) * DH, hB * DH:(hB + 1) * DH]
        wq = w_qkv[rows, :]               # [128, D]
        wk = w_qkv[D + rows, :]
        wv = w_qkv[2 * D + rows, :]
        in_maps.append({
            "xtb": xtb_prep,
            "wqt": _prep_chunked(np.ascontiguousarray(wq.T)).astype(ml_dtypes.bfloat16),
            "wkt": _prep_chunked(np.ascontiguousarray(wk.T)).astype(ml_dtypes.bfloat16),
            "wvt": _prep_chunked(np.ascontiguousarray(wv.T)).astype(ml_dtypes.bfloat16),
            "wot": wot_prep,
            "bb": bb_np,
        })

    last_results = bass_utils.run_bass_kernel_spmd(
        nc, in_maps, core_ids=list(range(N_CORES))
    )
    # core c's oc rows: [half k, 128] = global q rows 1024k + 128c + i
    arr = np.stack([last_results.results[c]["oc"] for c in range(N_CORES)])
    arr = arr.reshape(N_CORES, B, 2, 128, D).transpose(1, 2, 0, 3, 4)
    out = np.ascontiguousarray(arr.reshape(B, S, D))
    return out
